# revision 3
# baseline (speedup 1.0000x reference)
"""Trainium2 Bass kernel for nn_AlignModel.

Computes out[b, j, i] = sigmoid(simp[b,j]·w_s + orig[b,i]·w_o + bias) where
orig/simp are the two halves of prop_state[b] ([B, 2S, D] -> [B,S,D] each),
w_o = W[0,:D], w_s = W[0,D:].

Sharding: data-parallel over batch B=8 across the 8 NeuronCores. Each core:
  in  x   [4096, 512] f32  (= prop_state[b])
  in  w   [1, 1024]   f32
  in  bvec[1, 1]      f32
  out out [2048, 2048] f32 (= sigmoid(s_s[:,None] + s_o[None,:] + b))

Per-core pipeline:
  - broadcast w (and b) across partitions with a ones[1,128] matmul (PE)
  - s_o / s_s dot products: fused DVE tensor_tensor_reduce per [128,512] tile
  - transpose s_o [128,16] -> [16,128] (PE) and broadcast to PSUM [128,2048]
    via 16 rank-1 matmuls
  - per output row-tile t: ONE ScalarE activation
      out_t = Sigmoid(s_o_bcast + bias_col_t)   (PSUM -> SBUF)
    then a 1 MiB DMA to DRAM.
"""

import numpy as np

import concourse.mybir as mybir
from concourse import bacc, bass_utils
from concourse.masks import make_identity
from concourse.tile import TileContext

P = 128          # partitions
D = 512          # feature dim
S = 2048         # sents
NT = S // P      # 16 row tiles per half
GROUP = 4        # x tiles per input DMA (1 MiB chunks)
NG = NT // GROUP
NCORES = 8
F32 = mybir.dt.float32


def _kernel_body(tc, out, x, w, bvec):
    nc = tc.nc
    # rows r = n*128 + p  ->  [p, n, d]
    x_re = x.rearrange("(n p) d -> p n d", p=P)

    with (
        tc.tile_pool(name="consts", bufs=1) as cpool,
        tc.tile_pool(name="xin", bufs=3) as xpool,
        tc.tile_pool(name="scratch", bufs=2) as spool,
        tc.tile_pool(name="outbuf", bufs=4) as opool,
        tc.tile_pool(name="psum", bufs=1, space="PSUM") as ppool,
    ):
        # --- constants / setup ---
        ones_row = cpool.tile([1, P], F32, tag="ones")
        nc.gpsimd.memset(ones_row, 1.0)
        ident = cpool.tile([P, P], F32, tag="ident")
        make_identity(nc, ident)

        w_row = cpool.tile([1, 2 * D], F32, tag="wrow")
        nc.sync.dma_start(out=w_row, in_=w)
        b_sb = cpool.tile([1, 1], F32, tag="bsb")
        nc.sync.dma_start(out=b_sb, in_=bvec)

        # broadcast w across partitions: psum[j, d] = 1 * w[d]
        w_psum = ppool.tile([P, 2 * D], F32, tag="wpsum")
        nc.tensor.matmul(w_psum[:, 0:D], ones_row, w_row[:, 0:D],
                         start=True, stop=True)
        nc.tensor.matmul(w_psum[:, D:2 * D], ones_row, w_row[:, D:2 * D],
                         start=True, stop=True)
        w_bc = cpool.tile([P, 2 * D], F32, tag="wbc")
        nc.scalar.copy(w_bc, w_psum)

        s_o_mat = cpool.tile([P, NT], F32, tag="somat")    # s_o, col t = tile t
        s_sb_mat = cpool.tile([P, NT], F32, tag="ssmat")   # s_s

        # --- phase 1a: orig half -> s_o ---
        # dot product = DVE elementwise mul + ScalarE Copy with accum_out
        # (free-dim sum).
        for g in range(NG):
            xo = xpool.tile([P, GROUP, D], F32, tag="xchunk", name=f"xo{g}")
            nc.sync.dma_start(out=xo, in_=x_re[:, g * GROUP:(g + 1) * GROUP, :])
            for blk in range(GROUP):
                t = g * GROUP + blk
                prod = spool.tile([P, D], F32, tag="prod", name=f"po{t}")
                nc.vector.tensor_mul(out=prod, in0=xo[:, blk, :],
                                     in1=w_bc[:, 0:D])
                nc.scalar.activation(
                    prod, prod, mybir.ActivationFunctionType.Copy,
                    accum_out=s_o_mat[:, t:t + 1])

        # --- transpose s_o, flatten to a row (+b), broadcast via PE ---
        soT_psum = ppool.tile([NT, P], F32, tag="smallpsum")
        nc.tensor.transpose(soT_psum, s_o_mat, ident)
        soT = cpool.tile([NT, P], F32, tag="soT")
        nc.scalar.copy(soT, soT_psum)
        soT_row = cpool.tile([1, S], F32, tag="sorow")
        nc.sync.dma_start(out=soT_row, in_=soT)  # [16,128] -> [1,2048]
        nc.vector.tensor_scalar_add(soT_row, soT_row, b_sb)  # fold +b

        sob_psum = ppool.tile([P, S], F32, tag="sob")
        for t2 in range(S // 512):
            nc.tensor.matmul(sob_psum[:, t2 * 512:(t2 + 1) * 512], ones_row,
                             soT_row[:, t2 * 512:(t2 + 1) * 512],
                             start=True, stop=True)

        # --- phase 1b + 2: simp half -> s_s, then outputs ---
        for g in range(NG):
            xs = xpool.tile([P, GROUP, D], F32, tag="xchunk", name=f"xs{g}")
            nc.sync.dma_start(
                out=xs, in_=x_re[:, NT + g * GROUP:NT + (g + 1) * GROUP, :])
            for blk in range(GROUP):
                t = g * GROUP + blk
                prod = spool.tile([P, D], F32, tag="prod", name=f"ps{t}")
                nc.vector.tensor_mul(out=prod, in0=xs[:, blk, :],
                                     in1=w_bc[:, D:2 * D])
                nc.scalar.activation(
                    prod, prod, mybir.ActivationFunctionType.Copy,
                    accum_out=s_sb_mat[:, t:t + 1])
            for blk in range(GROUP):
                t = g * GROUP + blk
                o_sb = opool.tile([P, S], F32, tag="osb", name=f"ot{t}")
                nc.scalar.activation(
                    o_sb, sob_psum,
                    mybir.ActivationFunctionType.Sigmoid,
                    bias=s_sb_mat[:, t:t + 1],
                    scale=1.0,
                )
                nc.scalar.dma_start(out=out[t * P:(t + 1) * P, :], in_=o_sb)


def build_program():
    nc = bacc.Bacc(
        "TRN2",
        debug=False,
        target_bir_lowering=False,
        num_devices=NCORES,
    )
    x = nc.dram_tensor("x", [2 * S, D], F32, kind="ExternalInput").ap()
    w = nc.dram_tensor("w", [1, 2 * D], F32, kind="ExternalInput").ap()
    bvec = nc.dram_tensor("bvec", [1, 1], F32, kind="ExternalInput").ap()
    out = nc.dram_tensor("out", [S, S], F32, kind="ExternalOutput").ap()
    with TileContext(nc) as tc:
        _kernel_body(tc, out, x, w, bvec)
    nc.compile()
    return nc


_PROGRAM = None


def _get_program():
    global _PROGRAM
    if _PROGRAM is None:
        _PROGRAM = build_program()
    return _PROGRAM


def make_in_maps(prop_state, W, b):
    prop = np.ascontiguousarray(np.asarray(prop_state, dtype=np.float32))
    w = np.ascontiguousarray(np.asarray(W, dtype=np.float32).reshape(1, 2 * D))
    bv = np.ascontiguousarray(np.asarray(b, dtype=np.float32).reshape(1, 1))
    assert prop.shape == (NCORES, 2 * S, D), prop.shape
    return [{"x": prop[i], "w": w, "bvec": bv} for i in range(NCORES)]


def kernel(A, prop_state, W, b, _trace=False):
    nc = _get_program()
    in_maps = make_in_maps(prop_state, W, b)
    res = bass_utils.run_bass_kernel_spmd(
        nc, in_maps, core_ids=list(range(NCORES)), trace=_trace)
    out = np.stack([res.results[i]["out"] for i in range(NCORES)], axis=0)
    if _trace:
        kernel.last_results = res
    return out


# revision 4
# speedup vs baseline: 1.1572x; 1.1572x over previous
"""Trainium2 Bass kernel for nn_AlignModel.

Computes out[b, j, i] = sigmoid(simp[b,j]·w_s + orig[b,i]·w_o + bias) where
orig/simp are the two halves of prop_state[b] ([B, 2S, D] -> [B,S,D] each),
w_o = W[0,:D], w_s = W[0,D:].

Sharding: data-parallel over batch B=8 across the 8 NeuronCores. Each core:
  in  x   [4096, 512] f32  (= prop_state[b])
  in  w   [1, 1024]   f32
  in  bvec[1, 1]      f32
  out out [2048, 2048] f32 (= sigmoid(s_s[:,None] + (s_o + b)[None,:]))

Per-core pipeline (engine assignment tuned from the NTFF profile):
  - broadcast w (and b) across partitions with a ones[1,128] matmul (PE);
    the elementwise mults read w straight from PSUM
  - s_o (orig half): DVE tensor_mul + ScalarE Copy-with-accum reduce
    (ScalarE is idle in this phase; DVE stays mult-only)
  - s_o [128,16] -> transpose (PE) -> flatten [1,2048] (DMA) -> broadcast
    into PSUM [128,2048] via rank-1 matmuls, done in two halves so the
    chain overlaps the second half of phase 1
  - s_s (simp half): DVE tensor_mul + DVE tensor_reduce (+b per group),
    keeping ScalarE free for the sigmoids
  - per output row-tile t: ONE ScalarE op
      out_t = Sigmoid(s_o_bcast + bias_col_t)   (PSUM -> SBUF)
    then a 1 MiB DMA (Sync queue) to DRAM.
"""

import numpy as np

import concourse.mybir as mybir
from concourse import bacc, bass_utils
from concourse.masks import make_identity
from concourse.tile import TileContext

P = 128          # partitions
D = 512          # feature dim
S = 2048         # sents
NT = S // P      # 16 row tiles per half
GROUP = 4        # x tiles per input DMA (1 MiB chunks)
NG = NT // GROUP
NCORES = 8
F32 = mybir.dt.float32
HNT = NT // 2    # half of the row tiles (broadcast chain granularity)


def _kernel_body(tc, out, x, w, bvec):
    nc = tc.nc
    # rows r = n*128 + p  ->  [p, n, d]
    x_re = x.rearrange("(n p) d -> p n d", p=P)

    with (
        tc.tile_pool(name="consts", bufs=1) as cpool,
        tc.tile_pool(name="xin", bufs=3) as xpool,
        tc.tile_pool(name="scratch", bufs=4) as spool,
        tc.tile_pool(name="outbuf", bufs=4) as opool,
        tc.tile_pool(name="psum", bufs=1, space="PSUM") as ppool,
    ):
        # --- constants / setup ---
        ones_row = cpool.tile([1, P], F32, tag="ones")
        nc.gpsimd.memset(ones_row, 1.0)
        ident = cpool.tile([P, P], F32, tag="ident")
        make_identity(nc, ident)

        w_row = cpool.tile([1, 2 * D], F32, tag="wrow")
        nc.sync.dma_start(out=w_row, in_=w)
        b_sb = cpool.tile([1, 1], F32, tag="bsb")
        nc.sync.dma_start(out=b_sb, in_=bvec)

        # broadcast w across partitions: w_psum[j, d] = w[d]; mults read PSUM
        w_psum = ppool.tile([P, 2 * D], F32, tag="wpsum")
        nc.tensor.matmul(w_psum[:, 0:D], ones_row, w_row[:, 0:D],
                         start=True, stop=True)
        nc.tensor.matmul(w_psum[:, D:2 * D], ones_row, w_row[:, D:2 * D],
                         start=True, stop=True)

        # broadcast b across partitions -> [128, 1] (folded into s_s later)
        b_psum = ppool.tile([P, 1], F32, tag="smallpsum", bufs=2)
        nc.tensor.matmul(b_psum, ones_row, b_sb, start=True, stop=True)
        b_col = cpool.tile([P, 1], F32, tag="bcol")
        nc.scalar.copy(b_col, b_psum)

        s_o_mat = cpool.tile([P, NT], F32, tag="somat")    # s_o, col t = tile t
        s_sb_mat = cpool.tile([P, NT], F32, tag="ssmat")   # s_s + b
        sob_psum = ppool.tile([P, S], F32, tag="sob")      # bcast s_o rows

        def bcast_half(h):
            # transpose s_o cols [h*8, h*8+8) -> [8,128], flatten to a row,
            # then rank-1 matmuls into sob_psum[:, h*1024 : (h+1)*1024)
            soT_ps = ppool.tile([HNT, P], F32, tag="smallpsum", bufs=2,
                                name=f"soT_ps{h}")
            nc.tensor.transpose(soT_ps, s_o_mat[:, h * HNT:(h + 1) * HNT],
                                ident)
            soT = cpool.tile([HNT, P], F32, tag=f"soT{h}", name=f"soT{h}")
            nc.scalar.copy(soT, soT_ps)
            so_row = cpool.tile([1, HNT * P], F32, tag=f"sorow{h}",
                                name=f"so_row{h}")
            nc.sync.dma_start(out=so_row, in_=soT)
            for j in range(HNT * P // 512):
                o0 = h * HNT * P + j * 512
                nc.tensor.matmul(sob_psum[:, o0:o0 + 512], ones_row,
                                 so_row[:, j * 512:(j + 1) * 512],
                                 start=True, stop=True)

        # --- phase 1a: orig half -> s_o ---
        # dot product = DVE elementwise mul + ScalarE Copy with accum_out.
        for g in range(NG):
            xo = xpool.tile([P, GROUP, D], F32, tag="xchunk", name=f"xo{g}")
            nc.sync.dma_start(out=xo, in_=x_re[:, g * GROUP:(g + 1) * GROUP, :])
            for blk in range(GROUP):
                t = g * GROUP + blk
                prod = spool.tile([P, D], F32, tag="prod", name=f"po{t}")
                nc.vector.tensor_mul(out=prod, in0=xo[:, blk, :],
                                     in1=w_psum[:, 0:D])
                nc.scalar.activation(
                    prod, prod, mybir.ActivationFunctionType.Copy,
                    accum_out=s_o_mat[:, t:t + 1])
            if g == NG // 2 - 1:
                bcast_half(0)
        bcast_half(1)

        # --- phase 1b + 2: simp half -> s_s + b, then outputs ---
        for g in range(NG):
            xs = xpool.tile([P, GROUP, D], F32, tag="xchunk", name=f"xs{g}")
            nc.sync.dma_start(
                out=xs, in_=x_re[:, NT + g * GROUP:NT + (g + 1) * GROUP, :])
            for blk in range(GROUP):
                t = g * GROUP + blk
                prod = spool.tile([P, D], F32, tag="prod", name=f"ps{t}")
                nc.vector.tensor_mul(out=prod, in0=xs[:, blk, :],
                                     in1=w_psum[:, D:2 * D])
                nc.vector.tensor_reduce(
                    s_sb_mat[:, t:t + 1], prod,
                    axis=mybir.AxisListType.X, op=mybir.AluOpType.add)
            nc.vector.tensor_scalar_add(
                s_sb_mat[:, g * GROUP:(g + 1) * GROUP],
                s_sb_mat[:, g * GROUP:(g + 1) * GROUP], b_col)
            for blk in range(GROUP):
                t = g * GROUP + blk
                o_sb = opool.tile([P, S], F32, tag="osb", name=f"ot{t}")
                nc.scalar.activation(
                    o_sb, sob_psum,
                    mybir.ActivationFunctionType.Sigmoid,
                    bias=s_sb_mat[:, t:t + 1],
                    scale=1.0,
                )
                nc.sync.dma_start(out=out[t * P:(t + 1) * P, :], in_=o_sb)


def build_program():
    nc = bacc.Bacc(
        "TRN2",
        debug=False,
        target_bir_lowering=False,
        num_devices=NCORES,
    )
    x = nc.dram_tensor("x", [2 * S, D], F32, kind="ExternalInput").ap()
    w = nc.dram_tensor("w", [1, 2 * D], F32, kind="ExternalInput").ap()
    bvec = nc.dram_tensor("bvec", [1, 1], F32, kind="ExternalInput").ap()
    out = nc.dram_tensor("out", [S, S], F32, kind="ExternalOutput").ap()
    with TileContext(nc) as tc:
        _kernel_body(tc, out, x, w, bvec)
    nc.compile()
    return nc


_PROGRAM = None


def _get_program():
    global _PROGRAM
    if _PROGRAM is None:
        _PROGRAM = build_program()
    return _PROGRAM


def make_in_maps(prop_state, W, b):
    prop = np.ascontiguousarray(np.asarray(prop_state, dtype=np.float32))
    w = np.ascontiguousarray(np.asarray(W, dtype=np.float32).reshape(1, 2 * D))
    bv = np.ascontiguousarray(np.asarray(b, dtype=np.float32).reshape(1, 1))
    assert prop.shape == (NCORES, 2 * S, D), prop.shape
    return [{"x": prop[i], "w": w, "bvec": bv} for i in range(NCORES)]


def kernel(A, prop_state, W, b, _trace=False):
    nc = _get_program()
    in_maps = make_in_maps(prop_state, W, b)
    res = bass_utils.run_bass_kernel_spmd(
        nc, in_maps, core_ids=list(range(NCORES)), trace=_trace)
    out = np.stack([res.results[i]["out"] for i in range(NCORES)], axis=0)
    if _trace:
        kernel.last_results = res
    return out
